# revision 12
# baseline (speedup 1.0000x reference)
"""Multi-head attention Trainium2 Bass kernel.

Shapes (hardcoded): B=4, T=2048, E=1024, H=16, DK=64.
Sharding over 8 cores: core c -> (batch b = c//2, head-group g = c%2).
Each core computes 8 heads of one batch end-to-end and a partial output
projection; the host sums the two partials per batch.

Layout strategy (everything transposed so no on-device transposes):
  - inputs fed as x^T [E, T] (host-transposed)
  - Q^T, K^T kept as [f_local, T] (f on partitions)
  - V kept natural [T, f_local], stored per-head with an appended
    ones-column so attn@V also produces softmax row-sums in PSUM row 64
  - S^T [keys, q] per (head, key-tile); exp fused with 1/sqrt(dk) scale and
    additive mask bias (per-partition) on the scalar engine
  - output projection consumes x^T_local directly as lhsT
"""

import numpy as np

import concourse.bass as bass
import concourse.tile as tile
from concourse import bacc, mybir
from concourse.bass_utils import run_bass_kernel_spmd

F32 = mybir.dt.float32
F32R = mybir.dt.float32r

B, T, E, H = 4, 2048, 1024, 16
DK = E // H            # 64
N_CORES = 8
FL = 512               # local f (8 heads * 64)
HL = 8                 # heads per core
NT = T // 128          # 16 t-tiles
NE = E // 128          # 8 e-tiles
NFT = FL // 128        # 4 local f-tiles
NC4 = T // 512         # 4 t-chunks of 512

BF16 = mybir.dt.bfloat16
DT = BF16


def build_nc():
    nc = bacc.Bacc("TRN2", target_bir_lowering=False, debug=False,
                   enable_asserts=False)

    qT = nc.dram_tensor("qT", [E, T], DT, kind="ExternalInput").ap()
    kT = nc.dram_tensor("kT", [E, T], DT, kind="ExternalInput").ap()
    vT = nc.dram_tensor("vT", [E, T], DT, kind="ExternalInput").ap()
    wqT = nc.dram_tensor("wqT", [E, FL], DT, kind="ExternalInput").ap()
    wkT = nc.dram_tensor("wkT", [E, FL], DT, kind="ExternalInput").ap()
    wvT = nc.dram_tensor("wvT", [E, FL], DT, kind="ExternalInput").ap()
    woT = nc.dram_tensor("woT", [FL, E], DT, kind="ExternalInput").ap()
    bq = nc.dram_tensor("bq", [128, NFT], F32, kind="ExternalInput").ap()
    bk = nc.dram_tensor("bk", [128, NFT], F32, kind="ExternalInput").ap()
    bv = nc.dram_tensor("bv", [1, FL], DT, kind="ExternalInput").ap()
    bo = nc.dram_tensor("bo", [1, E], DT, kind="ExternalInput").ap()
    maskb = nc.dram_tensor("maskb", [128, NT], F32, kind="ExternalInput").ap()
    ones_d = nc.dram_tensor("ones_d", [1, 128], DT, kind="ExternalInput").ap()
    vones = nc.dram_tensor("vones", [128, HL], DT, kind="ExternalInput").ap()
    out = nc.dram_tensor("out", [T, E], F32, kind="ExternalOutput").ap()

    with tile.TileContext(nc) as tc:
        with (
            tc.tile_pool(name="const", bufs=1) as constp,
            tc.tile_pool(name="qkt", bufs=1) as qktp,
            tc.tile_pool(name="vsb", bufs=1) as vsbp,
            tc.tile_pool(name="xtl", bufs=1) as xtlp,
            tc.tile_pool(name="ps_s", bufs=2, space="PSUM") as ps_s,
            tc.tile_pool(name="ps_o", bufs=1, space="PSUM") as ps_o,
        ):
            # ---- constants ----
            bq_sb = constp.tile([128, NFT], F32, tag="bq")
            nc.sync.dma_start(out=bq_sb[:], in_=bq)
            bk_sb = constp.tile([128, NFT], F32, tag="bk")
            nc.sync.dma_start(out=bk_sb[:], in_=bk)
            bv_sb = constp.tile([1, FL], DT, tag="bv")
            nc.sync.dma_start(out=bv_sb[:], in_=bv)
            bo_sb = constp.tile([1, E], DT, tag="bo")
            nc.sync.dma_start(out=bo_sb[:], in_=bo)
            mask_sb = constp.tile([128, NT], F32, tag="maskb")
            nc.sync.dma_start(out=mask_sb[:], in_=maskb)
            ones_sb = constp.tile([1, 128], DT, tag="ones")
            nc.sync.dma_start(out=ones_sb[:], in_=ones_d)

            # persistent activation storage
            qt = [qktp.tile([128, T], DT, tag=f"qt{i}", name=f"qt{i}")
                  for i in range(NFT)]
            kt = [qktp.tile([128, T], DT, tag=f"kt{i}", name=f"kt{i}")
                  for i in range(NFT)]
            # V per t-tile: [128, 8 heads * 65]; col 64 of each head = 1.0
            vt = [vsbp.tile([128, HL * 65], DT, tag=f"v{j}", name=f"v{j}")
                  for j in range(NT)]
            xtl = [xtlp.tile([128, T], DT, tag=f"x{i}", name=f"x{i}")
                   for i in range(NFT)]

            # ---- phase 1a: V projection (natural layout + ones col) ----
            with tc.tile_pool(name="wv", bufs=1) as wvp, \
                 tc.tile_pool(name="vload", bufs=1) as vlp:
                wv_sb = [wvp.tile([128, FL], DT, tag=f"wv{e}", name=f"wv{e}")
                         for e in range(NE)]
                for e in range(NE):
                    nc.sync.dma_start(out=wv_sb[e][:],
                                      in_=wvT[e * 128:(e + 1) * 128, :])
                for hf in range(2):
                    vf = [vlp.tile([128, 1024], DT, tag=f"vf{e}",
                                   name=f"vf{e}") for e in range(NE)]
                    for e in range(NE):
                        nc.sync.dma_start(
                            out=vf[e][:],
                            in_=vT[e * 128:(e + 1) * 128,
                                   hf * 1024:(hf + 1) * 1024])
                    for jj in range(NT // 2):
                        j = hf * (NT // 2) + jj
                        ps = ps_s.tile([128, 1024], F32, tag="ps_s",
                                       name="psv")
                        for e in range(NE):
                            nc.tensor.matmul(
                                ps[:, 0:FL],
                                lhsT=vf[e][:, jj * 128:(jj + 1) * 128],
                                rhs=wv_sb[e][:],
                                start=(e == 0), stop=False)
                        nc.tensor.matmul(ps[:, 0:FL], lhsT=ones_sb[:],
                                         rhs=bv_sb[:], start=False, stop=True)
                        nc.sync.dma_start(
                            out=vt[j].rearrange(
                                "p (h w) -> p h w", w=65)[:, :, 64:65],
                            in_=vones.rearrange("p (h o) -> p h o", o=1))
                        nc.vector.tensor_copy(
                            out=vt[j].rearrange(
                                "p (h w) -> p h w", w=65)[:, :, 0:64],
                            in_=ps[:, 0:FL].rearrange(
                                "p (h w) -> p h w", w=64))

            # ---- phase 1b: K^T then Q^T projections ----
            with tc.tile_pool(name="wqk", bufs=1) as wp, \
                 tc.tile_pool(name="xload", bufs=10) as xlp:
                w_sb = {}
                for name, wdram in (("k", wkT), ("q", wqT)):
                    w_sb[name] = [
                        wp.tile([128, FL], DT, tag=f"w{name}{e}",
                                name=f"w{name}{e}") for e in range(NE)]
                    for e in range(NE):
                        nc.sync.dma_start(
                            out=w_sb[name][e][:],
                            in_=wdram[e * 128:(e + 1) * 128, :])

                for name, xdram, bias_sb, dst in (
                    ("k", kT, bk_sb, kt), ("q", qT, bq_sb, qt)):
                    for c in range(NC4):
                        xs = []
                        for e in range(NE):
                            xe = xlp.tile([128, 512], DT, tag="xchunk",
                                          name="xchunk")
                            nc.sync.dma_start(
                                out=xe[:],
                                in_=xdram[e * 128:(e + 1) * 128,
                                          c * 512:(c + 1) * 512])
                            xs.append(xe)
                        for fp in range(2):     # f-tile pairs
                            ps = ps_s.tile([128, 1024], F32, tag="ps_s",
                                           name="psqk")
                            for fi in range(2):
                                f = fp * 2 + fi
                                for e in range(NE):
                                    nc.tensor.matmul(
                                        ps[:, fi * 512:(fi + 1) * 512],
                                        lhsT=w_sb[name][e][:, f * 128:(f + 1) * 128],
                                        rhs=xs[e][:],
                                        start=(e == 0), stop=(e == NE - 1))
                            for fi in range(2):
                                f = fp * 2 + fi
                                nc.vector.tensor_scalar_add(
                                    dst[f][:, c * 512:(c + 1) * 512],
                                    ps[:, fi * 512:(fi + 1) * 512],
                                    bias_sb[:, f:f + 1])

            # ---- phase 2: attention per head ----
            with tc.tile_pool(name="exps", bufs=4) as expp, \
                 tc.tile_pool(name="norm", bufs=2) as normp:
                for h in range(HL):
                    po = h % 2 * 64
                    qh = qt[h // 2][po:po + 64, :]
                    kh = kt[h // 2][po:po + 64, :]
                    pso = ps_o.tile([65, T], F32, tag="ps_o", name="pso")
                    for k in range(NT):
                        for half in range(2):
                            pss = ps_s.tile([128, 1024], F32, tag="ps_s",
                                            name="pss")
                            for j in range(2):
                                c = half * 2 + j
                                nc.tensor.matmul(
                                    pss[:, j * 512:(j + 1) * 512],
                                    lhsT=kh[:, k * 128:(k + 1) * 128],
                                    rhs=qh[:, c * 512:(c + 1) * 512],
                                    start=True, stop=True)
                            es = expp.tile([128, 1024], DT, tag="es",
                                           name="es")
                            nc.scalar.activation(
                                out=es[:], in_=pss[:],
                                func=mybir.ActivationFunctionType.Exp,
                                bias=mask_sb[:, k:k + 1], scale=0.125)
                            for j in range(2):
                                c = half * 2 + j
                                nc.tensor.matmul(
                                    pso[0:65, c * 512:(c + 1) * 512],
                                    lhsT=vt[k][:, h * 65:h * 65 + 65],
                                    rhs=es[:, j * 512:(j + 1) * 512],
                                    start=(k == 0), stop=(k == NT - 1))
                    # normalize: rows 0..63 = O^T, row 64 = sum(exp)
                    ot = normp.tile([65, T], F32, tag="ot", name="ot")
                    nc.vector.tensor_copy(out=ot[:], in_=pso[0:65, :])
                    lg = normp.tile([1, T], F32, tag="lg", name="lg")
                    nc.scalar.activation(
                        out=lg[:], in_=ot[64:65, :],
                        func=mybir.ActivationFunctionType.Ln)
                    ri = normp.tile([1, T], F32, tag="ri", name="ri")
                    nc.scalar.activation(
                        out=ri[:], in_=lg[:],
                        func=mybir.ActivationFunctionType.Exp, scale=-1.0)
                    rep = normp.tile([64, T], F32, tag="rep", name="rep")
                    nc.sync.dma_start(out=rep[0:1, :], in_=ri[:])
                    for d in range(6):  # 1 -> 64 partitions
                        w = 1 << d
                        nc.sync.dma_start(out=rep[w:2 * w, :],
                                          in_=rep[0:w, :])
                    nc.vector.tensor_mul(xtl[h // 2][po:po + 64, :],
                                         ot[0:64, :], rep[:])

            # ---- phase 3: output projection (partial) ----
            with tc.tile_pool(name="wo", bufs=1) as wop, \
                 tc.tile_pool(name="osb", bufs=3) as osbp:
                wo_sb = [wop.tile([128, E], DT, tag=f"wo{e}", name=f"wo{e}")
                         for e in range(NFT)]
                for e in range(NFT):
                    nc.sync.dma_start(out=wo_sb[e][:],
                                      in_=woT[e * 128:(e + 1) * 128, :])
                for j in range(NT):
                    ps = ps_s.tile([128, E], F32, tag="ps_s", name="psf")
                    for e in range(NFT):
                        for c2 in range(2):
                            nc.tensor.matmul(
                                ps[:, c2 * 512:(c2 + 1) * 512],
                                lhsT=xtl[e][:, j * 128:(j + 1) * 128],
                                rhs=wo_sb[e][:, c2 * 512:(c2 + 1) * 512],
                                start=(e == 0), stop=False)
                    for c2 in range(2):
                        nc.tensor.matmul(
                            ps[:, c2 * 512:(c2 + 1) * 512],
                            lhsT=ones_sb[:],
                            rhs=bo_sb[:, c2 * 512:(c2 + 1) * 512],
                            start=False, stop=True)
                    ob = osbp.tile([128, E], F32, tag="ob", name="ob")
                    nc.vector.tensor_copy(out=ob[:], in_=ps[:])
                    nc.sync.dma_start(out=out[j * 128:(j + 1) * 128, :],
                                      in_=ob[:])

    nc.compile()
    return nc


_NC_CACHE = None


def _get_nc():
    global _NC_CACHE
    if _NC_CACHE is None:
        _NC_CACHE = build_nc()
    return _NC_CACHE


def make_in_maps(query, key_, value, mask, w_q, b_q, w_k, b_k, w_v, b_v,
                 w_o, b_o):
    import ml_dtypes
    f32 = np.float32
    bf16 = ml_dtypes.bfloat16
    c = lambda a: np.ascontiguousarray(a).astype(bf16)
    in_maps = []
    for core in range(N_CORES):
        b, g = core // 2, core % 2
        fs = slice(g * FL, (g + 1) * FL)
        mb = np.where(mask[b], 0.0, -30.0).astype(f32)
        in_maps.append({
            "qT": c(query[b].T.astype(f32, copy=False)),
            "kT": c(key_[b].T.astype(f32, copy=False)),
            "vT": c(value[b].T.astype(f32, copy=False)),
            "wqT": c(w_q[fs, :].T.astype(f32, copy=False)),
            "wkT": c(w_k[fs, :].T.astype(f32, copy=False)),
            "wvT": c(w_v[fs, :].T.astype(f32, copy=False)),
            "woT": c(w_o[:, fs].T.astype(f32, copy=False)),
            "bq": np.ascontiguousarray(
                b_q[fs].astype(f32, copy=False).reshape(NFT, 128).T),
            "bk": np.ascontiguousarray(
                b_k[fs].astype(f32, copy=False).reshape(NFT, 128).T),
            "bv": b_v[fs].reshape(1, FL).astype(bf16),
            "bo": (b_o.astype(f32, copy=False) if g == 0
                   else np.zeros(E, f32)).reshape(1, E).astype(bf16),
            "maskb": np.ascontiguousarray(mb.reshape(NT, 128).T),
            "ones_d": np.ones((1, 128), bf16),
            "vones": np.ones((128, HL), bf16),
        })
    return in_maps


def kernel(query=None, key_=None, value=None, mask=None, w_q=None, b_q=None,
           w_k=None, b_k=None, w_v=None, b_v=None, w_o=None, b_o=None,
           key=None, **_kwargs):
    if key_ is None:
        key_ = key
    args = [np.asarray(a) for a in
            (query, key_, value, mask, w_q, b_q, w_k, b_k, w_v, b_v,
             w_o, b_o)]
    nc = _get_nc()
    in_maps = make_in_maps(*args)
    res = run_bass_kernel_spmd(nc, in_maps, core_ids=list(range(N_CORES)))
    outs = [res.results[i]["out"] for i in range(N_CORES)]
    full = np.empty((B, T, E), np.float32)
    for b in range(B):
        full[b] = outs[2 * b] + outs[2 * b + 1]
    return full


# revision 14
# speedup vs baseline: 1.1590x; 1.1590x over previous
"""Multi-head attention Trainium2 Bass kernel.

Shapes (hardcoded): B=4, T=2048, E=1024, H=16, DK=64.
Sharding over 8 cores: core c -> (batch b = c//2, head-group g = c%2).
Each core computes 8 heads of one batch end-to-end and a partial output
projection; the host sums the two partials per batch.

Layout strategy (everything transposed so no on-device transposes):
  - inputs fed as x^T [E, T] (host-transposed)
  - Q^T, K^T kept as [f_local, T] (f on partitions)
  - V kept natural [T, f_local], stored per-head with an appended
    ones-column so attn@V also produces softmax row-sums in PSUM row 64
  - S^T [keys, q] per (head, key-tile); exp fused with 1/sqrt(dk) scale and
    additive mask bias (per-partition) on the scalar engine
  - output projection consumes x^T_local directly as lhsT
"""

import numpy as np

import concourse.bass as bass
import concourse.tile as tile
from concourse import bacc, mybir
from concourse.bass_utils import run_bass_kernel_spmd

F32 = mybir.dt.float32
F32R = mybir.dt.float32r

B, T, E, H = 4, 2048, 1024, 16
DK = E // H            # 64
N_CORES = 8
FL = 512               # local f (8 heads * 64)
HL = 8                 # heads per core
NT = T // 128          # 16 t-tiles
NE = E // 128          # 8 e-tiles
NFT = FL // 128        # 4 local f-tiles
NC4 = T // 512         # 4 t-chunks of 512

BF16 = mybir.dt.bfloat16
DT = BF16


def build_nc():
    nc = bacc.Bacc("TRN2", target_bir_lowering=False, debug=False,
                   enable_asserts=False)

    qT = nc.dram_tensor("qT", [E, T], DT, kind="ExternalInput").ap()
    kT = nc.dram_tensor("kT", [E, T], DT, kind="ExternalInput").ap()
    vT = nc.dram_tensor("vT", [E, T], DT, kind="ExternalInput").ap()
    wqT = nc.dram_tensor("wqT", [E, FL], DT, kind="ExternalInput").ap()
    wkT = nc.dram_tensor("wkT", [E, FL], DT, kind="ExternalInput").ap()
    wvT = nc.dram_tensor("wvT", [E, FL], DT, kind="ExternalInput").ap()
    woT = nc.dram_tensor("woT", [FL, E], DT, kind="ExternalInput").ap()
    bq = nc.dram_tensor("bq", [128, NFT], F32, kind="ExternalInput").ap()
    bk = nc.dram_tensor("bk", [128, NFT], F32, kind="ExternalInput").ap()
    bv = nc.dram_tensor("bv", [1, FL], DT, kind="ExternalInput").ap()
    bo = nc.dram_tensor("bo", [1, E], DT, kind="ExternalInput").ap()
    maskb = nc.dram_tensor("maskb", [128, NT], F32, kind="ExternalInput").ap()
    ones_d = nc.dram_tensor("ones_d", [1, 128], DT, kind="ExternalInput").ap()
    vones = nc.dram_tensor("vones", [128, HL], DT, kind="ExternalInput").ap()
    out = nc.dram_tensor("out", [T, E], F32, kind="ExternalOutput").ap()

    with tile.TileContext(nc) as tc:
        with (
            tc.tile_pool(name="const", bufs=1) as constp,
            tc.tile_pool(name="qkt", bufs=1) as qktp,
            tc.tile_pool(name="vsb", bufs=1) as vsbp,
            tc.tile_pool(name="xtl", bufs=1) as xtlp,
            tc.tile_pool(name="ps_s", bufs=2, space="PSUM") as ps_s,
            tc.tile_pool(name="ps_o", bufs=1, space="PSUM") as ps_o,
        ):
            # ---- constants ----
            bq_sb = constp.tile([128, NFT], F32, tag="bq")
            nc.sync.dma_start(out=bq_sb[:], in_=bq)
            bk_sb = constp.tile([128, NFT], F32, tag="bk")
            nc.sync.dma_start(out=bk_sb[:], in_=bk)
            bv_sb = constp.tile([1, FL], DT, tag="bv")
            nc.sync.dma_start(out=bv_sb[:], in_=bv)
            bo_sb = constp.tile([1, E], DT, tag="bo")
            nc.sync.dma_start(out=bo_sb[:], in_=bo)
            mask_sb = constp.tile([128, NT], F32, tag="maskb")
            nc.sync.dma_start(out=mask_sb[:], in_=maskb)
            ones_sb = constp.tile([1, 128], DT, tag="ones")
            nc.sync.dma_start(out=ones_sb[:], in_=ones_d)

            # persistent activation storage: per-head tiles, rows 0..63 =
            # head data, rows 64..127 = zeros (pad matmuls to full 128
            # contraction so the PE activity monitor keeps the clock warm)
            qt = [qktp.tile([128, T], DT, tag=f"qt{i}", name=f"qt{i}")
                  for i in range(HL)]
            kt = [qktp.tile([128, T], DT, tag=f"kt{i}", name=f"kt{i}")
                  for i in range(HL)]
            for i in range(HL):
                nc.vector.memset(qt[i][64:128, :], 0.0)
                nc.vector.memset(kt[i][64:128, :], 0.0)
            # V per t-tile: [128, 8 heads * 128]; per head: cols 0..63 = V,
            # col 64 = 1.0 (row-sum trick), cols 65..127 = zeros (padding)
            vt = [vsbp.tile([128, HL * 128], DT, tag=f"v{j}", name=f"v{j}")
                  for j in range(NT)]
            for j in range(NT):
                nc.vector.memset(
                    vt[j].rearrange("p (h w) -> p h w", w=128)[:, :, 65:128],
                    0.0)
            xtl = [xtlp.tile([128, T], DT, tag=f"x{i}", name=f"x{i}")
                   for i in range(NFT)]

            # ---- phase 1a: V projection (natural layout + ones col) ----
            with tc.tile_pool(name="wv", bufs=1) as wvp, \
                 tc.tile_pool(name="vload", bufs=1) as vlp:
                wv_sb = [wvp.tile([128, FL], DT, tag=f"wv{e}", name=f"wv{e}")
                         for e in range(NE)]
                for e in range(NE):
                    nc.sync.dma_start(out=wv_sb[e][:],
                                      in_=wvT[e * 128:(e + 1) * 128, :])
                for hf in range(2):
                    vf = [vlp.tile([128, 1024], DT, tag=f"vf{e}",
                                   name=f"vf{e}") for e in range(NE)]
                    for e in range(NE):
                        nc.sync.dma_start(
                            out=vf[e][:],
                            in_=vT[e * 128:(e + 1) * 128,
                                   hf * 1024:(hf + 1) * 1024])
                    for jj in range(NT // 2):
                        j = hf * (NT // 2) + jj
                        ps = ps_s.tile([128, 1024], F32, tag="ps_s",
                                       name="psv")
                        for e in range(NE):
                            nc.tensor.matmul(
                                ps[:, 0:FL],
                                lhsT=vf[e][:, jj * 128:(jj + 1) * 128],
                                rhs=wv_sb[e][:],
                                start=(e == 0), stop=False)
                        nc.tensor.matmul(ps[:, 0:FL], lhsT=ones_sb[:],
                                         rhs=bv_sb[:], start=False, stop=True)
                        nc.sync.dma_start(
                            out=vt[j].rearrange(
                                "p (h w) -> p h w", w=128)[:, :, 64:65],
                            in_=vones.rearrange("p (h o) -> p h o", o=1))
                        nc.vector.tensor_copy(
                            out=vt[j].rearrange(
                                "p (h w) -> p h w", w=128)[:, :, 0:64],
                            in_=ps[:, 0:FL].rearrange(
                                "p (h w) -> p h w", w=64))

            # ---- phase 1b: K^T then Q^T projections ----
            with tc.tile_pool(name="wqk", bufs=1) as wp, \
                 tc.tile_pool(name="xload", bufs=9) as xlp:
                w_sb = {}
                for name, wdram in (("k", wkT), ("q", wqT)):
                    w_sb[name] = [
                        wp.tile([128, FL], DT, tag=f"w{name}{e}",
                                name=f"w{name}{e}") for e in range(NE)]
                    for e in range(NE):
                        nc.sync.dma_start(
                            out=w_sb[name][e][:],
                            in_=wdram[e * 128:(e + 1) * 128, :])

                for name, xdram, bias_sb, dst in (
                    ("k", kT, bk_sb, kt), ("q", qT, bq_sb, qt)):
                    for c in range(NC4):
                        xs = []
                        for e in range(NE):
                            xe = xlp.tile([128, 512], DT, tag="xchunk",
                                          name="xchunk")
                            nc.sync.dma_start(
                                out=xe[:],
                                in_=xdram[e * 128:(e + 1) * 128,
                                          c * 512:(c + 1) * 512])
                            xs.append(xe)
                        for fp in range(2):     # f-tile pairs
                            ps = ps_s.tile([128, 1024], F32, tag="ps_s",
                                           name="psqk")
                            for fi in range(2):
                                f = fp * 2 + fi
                                for e in range(NE):
                                    nc.tensor.matmul(
                                        ps[:, fi * 512:(fi + 1) * 512],
                                        lhsT=w_sb[name][e][:, f * 128:(f + 1) * 128],
                                        rhs=xs[e][:],
                                        start=(e == 0), stop=(e == NE - 1))
                            for fi in range(2):
                                f = fp * 2 + fi
                                for hh in range(2):
                                    nc.vector.tensor_scalar_add(
                                        dst[2 * f + hh][0:64,
                                                        c * 512:(c + 1) * 512],
                                        ps[hh * 64:(hh + 1) * 64,
                                           fi * 512:(fi + 1) * 512],
                                        bias_sb[hh * 64:(hh + 1) * 64,
                                                f:f + 1])

            # ---- phase 2: attention per head ----
            with tc.tile_pool(name="exps", bufs=4) as expp, \
                 tc.tile_pool(name="norm", bufs=1) as normp:
                for h in range(HL):
                    qh = qt[h]
                    kh = kt[h]
                    pso = ps_o.tile([128, T], F32, tag="ps_o", name="pso")
                    for k in range(NT):
                        for half in range(2):
                            pss = ps_s.tile([128, 1024], F32, tag="ps_s",
                                            name="pss")
                            for j in range(2):
                                c = half * 2 + j
                                nc.tensor.matmul(
                                    pss[:, j * 512:(j + 1) * 512],
                                    lhsT=kh[:, k * 128:(k + 1) * 128],
                                    rhs=qh[:, c * 512:(c + 1) * 512],
                                    start=True, stop=True)
                            es = expp.tile([128, 1024], DT, tag="es",
                                           name="es")
                            nc.scalar.activation(
                                out=es[:], in_=pss[:],
                                func=mybir.ActivationFunctionType.Exp,
                                bias=mask_sb[:, k:k + 1], scale=0.125)
                            for j in range(2):
                                c = half * 2 + j
                                nc.tensor.matmul(
                                    pso[:, c * 512:(c + 1) * 512],
                                    lhsT=vt[k][:, h * 128:(h + 1) * 128],
                                    rhs=es[:, j * 512:(j + 1) * 512],
                                    start=(k == 0), stop=(k == NT - 1))
                    # normalize: rows 0..63 = O^T, row 64 = sum(exp)
                    ot = normp.tile([65, T], F32, tag="ot", name="ot")
                    nc.vector.tensor_copy(out=ot[:], in_=pso[0:65, :])
                    rep = normp.tile([64, T], F32, tag="rep", name="rep")
                    nc.scalar.activation(
                        out=rep[0:1, :], in_=ot[64:65, :],
                        func=mybir.ActivationFunctionType.Ln)
                    ri = normp.tile([1, T], F32, tag="ri", name="ri")
                    nc.scalar.activation(
                        out=ri[:], in_=rep[0:1, :],
                        func=mybir.ActivationFunctionType.Exp, scale=-1.0)
                    nc.sync.dma_start(out=rep[0:1, :], in_=ri[:])
                    for d in range(6):  # 1 -> 64 partitions
                        w = 1 << d
                        nc.sync.dma_start(out=rep[w:2 * w, :],
                                          in_=rep[0:w, :])
                    nc.vector.tensor_mul(
                        xtl[h // 2][h % 2 * 64:h % 2 * 64 + 64, :],
                        ot[0:64, :], rep[:])

            # ---- phase 3: output projection (partial) ----
            with tc.tile_pool(name="wo", bufs=1) as wop, \
                 tc.tile_pool(name="osb", bufs=2) as osbp:
                wo_sb = [wop.tile([128, E], DT, tag=f"wo{e}", name=f"wo{e}")
                         for e in range(NFT)]
                for e in range(NFT):
                    nc.sync.dma_start(out=wo_sb[e][:],
                                      in_=woT[e * 128:(e + 1) * 128, :])
                for j in range(NT):
                    ps = ps_s.tile([128, E], F32, tag="ps_s", name="psf")
                    for e in range(NFT):
                        for c2 in range(2):
                            nc.tensor.matmul(
                                ps[:, c2 * 512:(c2 + 1) * 512],
                                lhsT=xtl[e][:, j * 128:(j + 1) * 128],
                                rhs=wo_sb[e][:, c2 * 512:(c2 + 1) * 512],
                                start=(e == 0), stop=False)
                    for c2 in range(2):
                        nc.tensor.matmul(
                            ps[:, c2 * 512:(c2 + 1) * 512],
                            lhsT=ones_sb[:],
                            rhs=bo_sb[:, c2 * 512:(c2 + 1) * 512],
                            start=False, stop=True)
                    ob = osbp.tile([128, E], F32, tag="ob", name="ob")
                    nc.vector.tensor_copy(out=ob[:], in_=ps[:])
                    nc.sync.dma_start(out=out[j * 128:(j + 1) * 128, :],
                                      in_=ob[:])

    nc.compile()
    return nc


_NC_CACHE = None


def _get_nc():
    global _NC_CACHE
    if _NC_CACHE is None:
        _NC_CACHE = build_nc()
    return _NC_CACHE


def make_in_maps(query, key_, value, mask, w_q, b_q, w_k, b_k, w_v, b_v,
                 w_o, b_o):
    import ml_dtypes
    f32 = np.float32
    bf16 = ml_dtypes.bfloat16
    c = lambda a: np.ascontiguousarray(a).astype(bf16)
    in_maps = []
    for core in range(N_CORES):
        b, g = core // 2, core % 2
        fs = slice(g * FL, (g + 1) * FL)
        mb = np.where(mask[b], 0.0, -30.0).astype(f32)
        in_maps.append({
            "qT": c(query[b].T.astype(f32, copy=False)),
            "kT": c(key_[b].T.astype(f32, copy=False)),
            "vT": c(value[b].T.astype(f32, copy=False)),
            "wqT": c(w_q[fs, :].T.astype(f32, copy=False)),
            "wkT": c(w_k[fs, :].T.astype(f32, copy=False)),
            "wvT": c(w_v[fs, :].T.astype(f32, copy=False)),
            "woT": c(w_o[:, fs].T.astype(f32, copy=False)),
            "bq": np.ascontiguousarray(
                b_q[fs].astype(f32, copy=False).reshape(NFT, 128).T),
            "bk": np.ascontiguousarray(
                b_k[fs].astype(f32, copy=False).reshape(NFT, 128).T),
            "bv": b_v[fs].reshape(1, FL).astype(bf16),
            "bo": (b_o.astype(f32, copy=False) if g == 0
                   else np.zeros(E, f32)).reshape(1, E).astype(bf16),
            "maskb": np.ascontiguousarray(mb.reshape(NT, 128).T),
            "ones_d": np.ones((1, 128), bf16),
            "vones": np.ones((128, HL), bf16),
        })
    return in_maps


def kernel(query=None, key_=None, value=None, mask=None, w_q=None, b_q=None,
           w_k=None, b_k=None, w_v=None, b_v=None, w_o=None, b_o=None,
           key=None, **_kwargs):
    if key_ is None:
        key_ = key
    args = [np.asarray(a) for a in
            (query, key_, value, mask, w_q, b_q, w_k, b_k, w_v, b_v,
             w_o, b_o)]
    nc = _get_nc()
    in_maps = make_in_maps(*args)
    res = run_bass_kernel_spmd(nc, in_maps, core_ids=list(range(N_CORES)))
    outs = [res.results[i]["out"] for i in range(N_CORES)]
    full = np.empty((B, T, E), np.float32)
    for b in range(B):
        full[b] = outs[2 * b] + outs[2 * b + 1]
    return full
